# revision 90
# baseline (speedup 1.0000x reference)
"""Trainium2 Bass kernel for nn_MultiHeadAttention (SL=2048, BS=2, D=1024, H=16, DH=64).

Sharding: the [BS=2, H=16] grid of attention heads is split across 8 cores:
core c handles batch b = c//4 and heads 4*(c%4) .. 4*(c%4)+4.
Each core computes q/k/v projections for its own head slice, the 4 attention
maps, and a partial output (its heads' contribution through Wo). The host
sums the 4 partials per batch.

Scores are computed transposed (S^T[k, q]) so softmax-exp output feeds the
AV matmul directly; softmax denominators come from a ones-column appended to
V (row 64 of the AV accumulator), reciprocal'd and broadcast across 64
partitions with a rank-1 matmul so normalization is one elementwise multiply.

q/k are stored fp16 so the score matmuls use the fast LDWEIGHTS path (f32r
weights would fall into the slow self-loading matmul). AV consumption lags
the score/exp pipeline by 2 key-tiles to hide scalar-engine latency. All
inputs are DMA'd up front in a token-chunk-major layout (8KB contiguous
lines per partition).
"""

import os
import numpy as np

SL, BS, D = 2048, 2, 1024
H, DH = 16, 64
NCORES = 8
HPC = 4            # heads per core
OD = HPC * DH      # 256 projected dims per core
DC = D // 128      # 8 contraction chunks
QC = SL // 512     # 4 query/token chunks of 512
KT = SL // 128     # 16 key tiles of 128
XCH = DC * 512     # free size of one resident x chunk

_NC = None
LAST_RESULT = None

# Schraudolph exp in bf16 bit-space: bf16_bits(exp(x)) ~= round(A*x + B).
# A = 2^7/ln2; B = 127*2^7 - 7.78 centers the (1+f)/2^f sawtooth (max rel
# err ~4.1%). Key tiles in DVE_KT get this approximation on the vector
# engine; the rest use the scalar engine's exact exp. The softmax
# denominator sums the same approximate values, so the common-mode error
# cancels.
SCHRA_A = 184.6650292180933
SCHRA_B = 16248.22
DVE_KT = (2, 5, 8, 11, 15)


def _build_nc():
    import concourse.mybir as mybir
    import concourse.tile as tile
    from concourse import bacc

    f32 = mybir.dt.float32
    f32r = mybir.dt.float32r
    bf16 = mybir.dt.bfloat16
    f16 = mybir.dt.float16
    EXP = mybir.ActivationFunctionType.Exp

    nc = bacc.Bacc(None, target_bir_lowering=False, debug=True)

    # x tensors in [p, cc, d, s] layout: row p holds, for each token chunk cc,
    # the 8 D-chunks' 512-token rows contiguously (8KB DMA lines).
    xqT = nc.dram_tensor("xqT", [128, QC * XCH], f16, kind="ExternalInput")
    xkT = nc.dram_tensor("xkT", [128, QC * XCH], f16, kind="ExternalInput")
    xvT = nc.dram_tensor("xvT", [128, QC * XCH], f16, kind="ExternalInput")
    wqT = nc.dram_tensor("wqT", [128, DC * OD], f16, kind="ExternalInput")
    wkT = nc.dram_tensor("wkT", [128, DC * OD], f16, kind="ExternalInput")
    wvT = nc.dram_tensor("wvT", [128, DC * OD], f16, kind="ExternalInput")
    woT = nc.dram_tensor("woT", [128, 2 * D], f16, kind="ExternalInput")
    onesd = nc.dram_tensor("onesd", [128, 260], f32r, kind="ExternalInput")
    # output in [p, qc, o8, s] chunk-major layout (8KB DMA lines)
    yT = nc.dram_tensor("yT", [128, QC * 8 * 512], f16, kind="ExternalOutput")
    debug = bool(int(os.environ.get("KERNEL_DEBUG", "0")))
    if debug:
        dbg_qT = [nc.dram_tensor(f"dbg_qT{i}", [128, 512], f16, kind="ExternalOutput") for i in range(2)]
        dbg_kT = [nc.dram_tensor(f"dbg_kT{i}", [128, SL], f16, kind="ExternalOutput") for i in range(2)]
        dbg_v = [nc.dram_tensor(f"dbg_v{i}", [128, 260], bf16, kind="ExternalOutput") for i in range(16)]
        dbg_E = nc.dram_tensor("dbg_E", [128, 1024], bf16, kind="ExternalOutput")
        dbg_avs = [nc.dram_tensor(f"dbg_avs{i}", [65, 512], f32, kind="ExternalOutput") for i in range(2)]
        dbg_rec = [nc.dram_tensor(f"dbg_rec{i}", [1, 512], f32, kind="ExternalOutput") for i in range(2)]
        dbg_BC = [nc.dram_tensor(f"dbg_BC{i}", [64, 512], f32, kind="ExternalOutput") for i in range(2)]
        dbg_OT = nc.dram_tensor("dbg_OT", [128, 512], f16, kind="ExternalOutput")

    with tile.TileContext(nc) as tc:
        with (
            tc.tile_pool(name="wsb", bufs=1) as wsb,
            tc.tile_pool(name="qk", bufs=1) as qk,
            tc.tile_pool(name="vsb", bufs=1) as vsb,
            tc.tile_pool(name="xsb", bufs=1) as xsb,
            tc.tile_pool(name="esb", bufs=6) as esb,
            tc.tile_pool(name="rsb", bufs=2) as rsb,
            tc.tile_pool(name="otsb", bufs=4) as otsb,
            tc.tile_pool(name="ysb", bufs=3) as ysb,
            tc.tile_pool(name="otmp", bufs=2) as otmp,
            tc.tile_pool(name="avsb", bufs=2) as avsb,
            tc.tile_pool(name="ybig", bufs=2) as ybig,
            tc.tile_pool(name="pp", bufs=1, space="PSUM") as pp,
            tc.tile_pool(name="wp", bufs=2, space="PSUM") as wp,
            tc.tile_pool(name="avop", bufs=1, space="PSUM") as avop,
            tc.tile_pool(name="yp", bufs=1, space="PSUM") as yp,
        ):
            # --- persistent SBUF tensors ---
            wq_sb = wsb.tile([128, DC * OD], f16, tag="wq")  # [p, dc*256+od]
            wk_sb = wsb.tile([128, DC * OD], f16, tag="wk")
            wv_sb = wsb.tile([128, DC * OD], f16, tag="wv")
            wo_sb = wsb.tile([128, 2 * D], f16, tag="wo")    # [p, hp*1024+o]
            ones_sb = wsb.tile([128, 260], f16, tag="ones")
            ones_bb = wsb.tile([65, 64], bf16, tag="onesb")
            kT_sb = [qk.tile([128, SL], f16, tag=f"kT{ot}", name=f"kT{ot}") for ot in range(2)]
            qT_sb = [qk.tile([128, SL], f16, tag=f"qT{ot}", name=f"qT{ot}") for ot in range(2)]
            v_sb = [vsb.tile([128, 260], bf16, tag=f"v{t}", name=f"v{t}") for t in range(KT)]
            # x chunks are split into d-halves (separate tiles) so the first
            # projection matmuls start as soon as half a chunk has landed
            HXCH = XCH // 2
            xk_sb = {(c, h): xsb.tile([128, HXCH], f16, tag=f"xk{c}_{h}",
                                      name=f"xk{c}_{h}")
                     for c in range(QC) for h in range(2)}
            xq_sb = {(c, h): xsb.tile([128, HXCH], f16, tag=f"xq{c}_{h}",
                                      name=f"xq{c}_{h}")
                     for c in range(QC) for h in range(2)}
            xv_sb = {(c, h): xsb.tile([128, HXCH], f16, tag=f"xv{c}_{h}",
                                      name=f"xv{c}_{h}")
                     for c in range(QC) for h in range(2)}

            def load_w(dst, src):
                nc.sync.dma_start(out=dst[:], in_=src[:])

            def load_x(dst_tiles, xdram, cc):
                for h in range(2):
                    nc.sync.dma_start(
                        out=dst_tiles[(cc, h)][:],
                        in_=xdram[:, cc * XCH + h * HXCH:
                                  cc * XCH + (h + 1) * HXCH])

            def xsl(tiles, cc, d):
                # [128, 512] slice of token-chunk cc, D-chunk d
                return tiles[(cc, d // 4)][:, (d % 4) * 512:(d % 4 + 1) * 512]

            # --- input DMAs, issued up front in dependency order; the
            # non-critical chunks are gated on projection progress (via tiny
            # gpsimd touches) so they don't steal HBM bandwidth from the
            # startup-critical loads ---
            # spread the issue cost of the startup-critical loads across
            # otherwise-idle engine queues (each DMA trigger costs ~600ns on
            # its issuing engine)
            # ones_sb only feeds the warm-up matmuls now (the denominator
            # broadcast uses the bf16 memset tile), so memset it instead of
            # DMA: the warm-ups start immediately instead of waiting on HBM
            nc.gpsimd.memset(ones_sb[:], 1.0)
            load_w(wk_sb, wkT)
            for cc in range(QC):
                load_x(xk_sb, xkT, cc)
            load_w(wq_sb, wqT)
            load_x(xq_sb, xqT, 0)
            load_w(wv_sb, wvT)
            load_x(xv_sb, xvT, 0)
            load_w(wo_sb, woT)

            def gate_x(dst_tiles, xdram, cc, gate_ap):
                # WAR chain: touch the tile from gate_ap, making the DMA
                # wait until the gating tile has been produced; issue from
                # gpsimd so the touch and the trigger share a queue
                for h in range(2):
                    nc.gpsimd.tensor_copy(dst_tiles[(cc, h)][:, 0:1], gate_ap)
                    nc.gpsimd.dma_start(
                        out=dst_tiles[(cc, h)][:],
                        in_=xdram[:, cc * XCH + h * HXCH:
                                  cc * XCH + (h + 1) * HXCH])

            # ones columns of the v tiles (positions 64, 129, 194, 259) are
            # written once here; the per-tile projection copy skips them
            for t in range(KT):
                nc.gpsimd.memset(v_sb[t][:], 1.0)
            nc.gpsimd.memset(ones_bb[:], 1.0)

            qp_ps = {}

            def proj_qk_quarter(w_sb, dst, xtiles, cc, ot, half, pool=None):
                if half == 0:
                    pl, tg = (pool if pool is not None else (pp, "pp"))
                    qp_ps[(cc, ot)] = pl.tile([128, 512], f32, tag=tg,
                                              name="ps")
                ps = qp_ps[(cc, ot)]
                for d in range(half * 4, half * 4 + 4):
                    nc.tensor.matmul(
                        ps[:],
                        (w_sb[:, d * OD + ot * 128: d * OD + (ot + 1) * 128]),
                        (xsl(xtiles, cc, d)),
                        start=(d == 0), stop=(d == DC - 1))
                if half == 1:
                    nc.vector.tensor_copy(dst[ot][:, cc * 512:(cc + 1) * 512],
                                          ps[:])

            def proj_qk(w_sb, dst, xtiles, cc, alt=False):
                # during startup, ping-pong the accumulator between the pp and
                # yp pools so the PSUM->SBUF evacuation overlaps the next
                # quarter's matmuls instead of stalling on the single buffer.
                # half-major order: both ot's half-0 quarters run on the
                # chunk's first half-DMA before the second half is needed
                for half in range(2):
                    for ot in range(2):
                        pool = (yp, "yp") if (alt and ot == 1) else (pp, "pp")
                        proj_qk_quarter(w_sb, dst, xtiles, cc, ot, half, pool)

            WoY = {}

            def emit_wo_piece(qc_, ot_tiles, pool, ptag, o8):
                if o8 == 0:
                    WoY[qc_] = ybig.tile([128, 8 * 512], f16, tag="ybig",
                                         name="ybig")
                Y = pool.tile([128, 512], f32, tag=ptag, name="Y")
                for hp in range(2):
                    nc.tensor.matmul(
                        Y[:],
                        (wo_sb[:, hp * D + o8 * 128: hp * D + (o8 + 1) * 128]),
                        (ot_tiles[hp][:]),
                        start=(hp == 0), stop=(hp == 1))
                # alternate cast engine so casts keep pace with the matmuls
                if qc_ == QC - 1 and o8 % 2:
                    nc.scalar.copy(WoY[qc_][:, o8 * 512:(o8 + 1) * 512], Y[:])
                else:
                    nc.vector.tensor_copy(
                        WoY[qc_][:, o8 * 512:(o8 + 1) * 512], Y[:])
                if qc_ == QC - 1:
                    # tail: per-piece DMAs start transferring as soon as each
                    # piece is cast and overlap each other across queues
                    nc.sync.dma_start(
                        out=yT[:, qc_ * 4096 + o8 * 512:
                               qc_ * 4096 + (o8 + 1) * 512],
                        in_=WoY[qc_][:, o8 * 512:(o8 + 1) * 512])
                elif o8 == 7:
                    nc.sync.dma_start(
                        out=yT[:, qc_ * 4096:(qc_ + 1) * 4096],
                        in_=WoY[qc_][:])

            def emit_wo(qc_, ot_tiles, pools):
                for o8 in range(8):
                    pool, ptag = pools[o8 % len(pools)]
                    emit_wo_piece(qc_, ot_tiles, pool, ptag, o8)

            # --- warm-up matmuls (HAM un-throttle) ---
            warm = yp.tile([128, 512], f32, tag="yp", name="warm")
            for i in range(24):
                nc.tensor.matmul(warm[0:64, 0:256], ones_sb[:, 0:64],
                                 ones_sb[:, 0:256], start=(i == 0),
                                 stop=(i == 23))
            warms = ysb.tile([64, 256], f32, tag="ys", name="warms")
            nc.vector.tensor_copy(warms[:], warm[0:64, 0:256])

            # --- k projection (all chunks), q projection chunk 0 ---
            for cc in range(QC):
                proj_qk(wk_sb, kT_sb, xk_sb, cc, alt=True)
            proj_qk(wq_sb, qT_sb, xq_sb, 0, alt=True)

            # non-critical input chunks, gated on projection progress
            gate_x(xq_sb, xqT, 1, kT_sb[0][:, 512:513])
            gate_x(xv_sb, xvT, 1, kT_sb[0][:, 1024:1025])
            gate_x(xq_sb, xqT, 2, kT_sb[0][:, 1536:1537])
            gate_x(xv_sb, xvT, 2, qT_sb[0][:, 0:1])
            gate_x(xq_sb, xqT, 3, qT_sb[0][:, 1:2])
            gate_x(xv_sb, xvT, 3, qT_sb[0][:, 2:3])

            def emit_vtile(t_):
                cc_, tt = divmod(t_, 4)
                ps = pp.tile([128, OD], f32, tag="pp", name="ps")
                for d in range(DC):
                    xs = xsl(xv_sb, cc_, d)
                    nc.tensor.matmul(
                        ps[:],
                        (xs[:, tt * 128:(tt + 1) * 128]),
                        (wv_sb[:, d * OD:(d + 1) * OD]),
                        start=(d == 0), stop=(d == DC - 1))
                # one strided copy into the 4 x 64 head slots, skipping the
                # pre-set ones columns at 64/129/194/259; alternate engines
                # so ladder 0's vector queue (which also runs the Schraudolph
                # exps that free score PSUM buffers) doesn't clog
                dst = v_sb[t_][:, 0:260].rearrange(
                    "p (h x) -> p h x", h=4, x=65)[:, :, 0:64]
                src = ps[:].rearrange("p (h x) -> p h x", h=4, x=64)
                if t_ % 2:
                    nc.scalar.copy(dst, src)
                else:
                    nc.vector.tensor_copy(dst, src)

            # --- attention: 8 ladders (qc-major, head-pair minor), with
            # fine-grained insertions so ACT stays saturated ---
            inserts = {}

            def at(L_, kt_, fn):
                inserts.setdefault((L_, kt_), []).append(fn)

            OTs = {}

            # dependency-free filler matmuls in ladder 0's early slots: the
            # exp-pipeline fill stalls the PE ~2.7us there, which trips the
            # HAM idle window and halves the PE clock for ~10us; these keep
            # the activity monitor busy through the stall
            def l0_fill():
                wf = yp.tile([64, 256], f32, tag="yp", name="wf")
                for i in range(4):
                    nc.tensor.matmul(wf[:], ones_sb[:, 0:64],
                                     ones_sb[:, 0:256], start=True, stop=True)
            for sl_ in range(4):
                at(0, sl_, l0_fill)

            # schedule q-projections (per-ot quarters) for qc 1..3
            for qc_ in range(1, QC):
                Lt = 1 if qc_ == 1 else (qc_ - 1) * 2
                for j in range(4):
                    at(Lt, 5 + 2 * j,
                       (lambda q=qc_, ot=j // 2, hf=j % 2:
                        proj_qk_quarter(wq_sb, qT_sb, xq_sb, q, ot, hf)))

            for L in range(2 * QC):
                qc, hp = divmod(L, 2)
                AVO = [avop.tile([65, 512], f32, tag=f"av{hip}", name="AVO")
                       for hip in range(2)]

                def emit_av(E_, kt_, AVO=AVO, hp=hp):
                    for hip in range(2):
                        nc.tensor.matmul(
                            AVO[hip][:],
                            (v_sb[kt_][:, (hp * 2 + hip) * 65:
                                         (hp * 2 + hip) * 65 + 65]),
                            (E_[:, hip * 512:(hip + 1) * 512]),
                            start=(kt_ == 0), stop=(kt_ == KT - 1))

                # on the final ladder, put the last key tiles' exps on the
                # vector engine too, so the scalar engine's queue drains
                # before the tail chains need the accumulators
                dve_set = DVE_KT if L < 2 * QC - 1 else (2, 5, 8, 11, 13, 14, 15)
                Eq = {}
                for kt in range(KT):
                    W = wp.tile([128, 1024], f32, tag="wp", name="W")
                    for hip in range(2):
                        nc.tensor.matmul(
                            W[:, hip * 512:(hip + 1) * 512],
                            (kT_sb[hp][hip * 64:(hip + 1) * 64,
                                         kt * 128:(kt + 1) * 128]),
                            (qT_sb[hp][hip * 64:(hip + 1) * 64,
                                         qc * 512:(qc + 1) * 512]),
                            start=True, stop=True)
                    if kt in dve_set:
                        Ei = esb.tile([128, 1024], mybir.dt.uint16, tag="E",
                                      name="Ei")
                        nc.vector.tensor_scalar(
                            out=Ei[:], in0=W[:], scalar1=SCHRA_A,
                            scalar2=SCHRA_B, op0=mybir.AluOpType.mult,
                            op1=mybir.AluOpType.add)
                        Eq[kt] = Ei[:].bitcast(bf16)
                    else:
                        E = esb.tile([128, 1024], bf16, tag="E", name="E")
                        nc.scalar.activation(E[:], W[:], EXP)
                        Eq[kt] = E[:]
                        if debug and L == 0 and kt == 0:
                            nc.sync.dma_start(out=dbg_E[:], in_=E[:])
                    # kt-pairing: two score pairs run back-to-back (the next
                    # pair's weight loads pull ahead into the background
                    # buffer during the current pair's stream), then the two
                    # lagged AV pairs — halving the exposed scores<->AV
                    # weight-load transitions
                    if kt % 2 == 1:
                        for kt_ in (kt - 4, kt - 3):
                            if kt_ >= 0:
                                emit_av(Eq.pop(kt_), kt_)
                    if L == 0:
                        emit_vtile(kt)
                    for fn in inserts.pop((L, kt), []):
                        fn()
                for ktf in (KT - 3, KT - 2, KT - 1):
                    emit_av(Eq.pop(ktf), ktf)

                # evacuate accumulators promptly, then defer the normalize
                # chain into the next ladder
                last_ladder = (L == 2 * QC - 1)
                avs_pair = {}

                def evac(hip):
                    avs = avsb.tile([65, 512], f32, tag="avs", name="avs")
                    if last_ladder:
                        # tail only: scalar engine is idle after its last exp
                        nc.scalar.copy(avs[:], AVO[hip][:])
                    else:
                        nc.vector.tensor_copy(avs[:], AVO[hip][:])
                    avs_pair[hip] = avs
                    if debug and L == 0:
                        nc.sync.dma_start(out=dbg_avs[hip][:], in_=avs[:])

                if not last_ladder:
                    evac(0)
                    evac(1)
                if debug and L == 0:
                    for i in range(2):
                        nc.sync.dma_start(out=dbg_qT[i][:], in_=qT_sb[i][:, 0:512])
                        nc.sync.dma_start(out=dbg_kT[i][:], in_=kT_sb[i][:, 0:SL])
                    for i in range(16):
                        nc.sync.dma_start(out=dbg_v[i][:], in_=v_sb[i][:])
                OT = otsb.tile([128, 512], f16, tag="ot", name="OT")
                OTs[(qc, hp)] = OT

                def chain(hip, avs_pair=avs_pair, OT=OT, L=L):
                    avs = avs_pair[hip]
                    sums_b = rsb.tile([65, 512], bf16, tag="recip",
                                      name="sums_b")
                    if L == 2 * QC - 1:
                        nc.scalar.copy(sums_b[64:65, :], avs[64:65, :])
                    else:
                        nc.vector.tensor_copy(sums_b[64:65, :], avs[64:65, :])
                    BCp = pp.tile([64, 512], f32, tag="pp", name="BCp")
                    nc.tensor.matmul(BCp[:], ones_bb[64:65, :],
                                     sums_b[64:65, :], start=True, stop=True)
                    BCs = rsb.tile([64, 512], f32, tag="recr", name="BCs")
                    nc.vector.reciprocal_approx_fast(BCs[:], BCp[:])
                    if debug and L == 0:
                        nc.sync.dma_start(out=dbg_rec[hip][:],
                                          in_=BCs[0:1, :])
                        nc.sync.dma_start(out=dbg_BC[hip][:], in_=BCs[:])
                    if hip == 0:
                        nc.vector.tensor_mul(OT[0:64, :], avs[0:64, :],
                                             BCs[:])
                    else:
                        OTt = otmp.tile([64, 512], f16, tag="otmp",
                                        name="OTt")
                        nc.vector.tensor_mul(OTt[:], avs[0:64, :], BCs[:])
                        nc.sync.dma_start(out=OT[64:128, :], in_=OTt[:])
                        if debug and L == 0:
                            nc.sync.dma_start(out=dbg_OT[:], in_=OT[:])

                if not last_ladder:
                    at(L + 1, 1, (lambda c=chain: c(0)))
                    at(L + 1, 3, (lambda c=chain: c(1)))
                else:
                    # hip1 first: its OTt->OT partition-merge DMA overlaps
                    # hip0's chain, shortening the tail critical path
                    evac(1)
                    chain(1)
                    evac(0)
                    chain(0)

                # spread Wo(qc) pieces across the NEXT hp==1 ladder
                if hp == 1 and qc < QC - 1:
                    for o8 in range(8):
                        at(L + 2, 4 + o8,
                           (lambda q=qc, o=o8:
                            emit_wo_piece(q, [OTs[(q, 0)], OTs[(q, 1)]],
                                          yp, 'yp', o)))

            emit_wo(QC - 1, [OTs[(QC - 1, 0)], OTs[(QC - 1, 1)]],
                    [(yp, 'yp'), (avop, 'av0'), (avop, 'av1'), (pp, 'pp')])

    nc.compile()
    return nc


def _get_nc():
    global _NC
    if _NC is None:
        _NC = _build_nc()
    return _NC


def _host_fallback(query, keys, values, mask, Wq, Wk, Wv, Wo):
    # Exact reference math in numpy; only used if mask has zeros (off-spec).
    q = (query @ Wq.T).reshape(SL, BS, H, DH)
    k = (keys @ Wk.T).reshape(SL, BS, H, DH)
    v = (values @ Wv.T).reshape(SL, BS, H, DH)
    out = np.zeros((SL, BS, H * DH), np.float32)
    for b in range(BS):
        for h in range(H):
            s = q[:, b, h, :] @ k[:, b, h, :].T
            s = np.where(mask[0, 0] == 0, np.float32(-1e20), s)
            s = s - s.max(axis=-1, keepdims=True)
            p = np.exp(s)
            p /= p.sum(axis=-1, keepdims=True)
            out[:, b, h * DH:(h + 1) * DH] = p @ v[:, b, h, :]
    return out @ Wo.T


def _enable_trace_support():
    """Install the antenv.axon_hooks shim so trace=True works under axon."""
    import sys
    import types
    import antenv
    if "antenv.axon_hooks" in sys.modules:
        return
    hookmod = types.ModuleType("antenv.axon_hooks")
    _hook = [None]
    hookmod.set_axon_ntff_profile_hook = lambda h: _hook.__setitem__(0, h)
    hookmod.get_axon_ntff_profile_hook = lambda: _hook[0]
    antenv.axon_hooks = hookmod
    sys.modules["antenv.axon_hooks"] = hookmod
    try:
        from trn_agent_boot.trn_boot import _ntff_profile_via_ctypes
        hookmod.set_axon_ntff_profile_hook(
            _ntff_profile_via_ctypes("/opt/axon/libaxon_pjrt.so"))
    except Exception:
        pass
    import concourse.bass_utils as bu
    bu.upload_artifacts = lambda tmpdir: tmpdir


def _w_sb_layout(Wslice):
    # [256 od, 1024 D] -> [128 p, dc*256+od]
    return np.ascontiguousarray(
        Wslice.reshape(OD, DC, 128).transpose(2, 1, 0).reshape(128, DC * OD))


def _wo_sb_layout(WoSlice):
    # [1024 o, 256 hd] -> [128 p, hp*1024+o]
    return np.ascontiguousarray(
        WoSlice.reshape(D, 2, 128).transpose(2, 1, 0).reshape(128, 2 * D))


def _x_layout(x2d):
    # x2d: [SL, D] for one batch -> [128 p, cc, d, s] flattened, so each
    # (p, cc) pair's 8 D-chunk rows are contiguous 8KB DMA lines
    a = np.asarray(x2d, np.float32).T.reshape(DC, 128, QC, 512)
    return np.ascontiguousarray(
        a.transpose(1, 2, 0, 3).reshape(128, QC * DC * 512)).astype(np.float16)


def kernel(query, keys, values, mask, Wq, Wk, Wv, Wo):
    query = np.asarray(query, np.float32)
    keys = np.asarray(keys, np.float32)
    values = np.asarray(values, np.float32)
    mask = np.asarray(mask)
    Wq = np.asarray(Wq, np.float32)
    Wk = np.asarray(Wk, np.float32)
    Wv = np.asarray(Wv, np.float32)
    Wo = np.asarray(Wo, np.float32)

    if (mask == 0).any():
        return _host_fallback(query, keys, values, mask, Wq, Wk, Wv, Wo)

    trace = bool(int(os.environ.get("KERNEL_TRACE", "0")))
    if trace:
        _enable_trace_support()

    from concourse.bass_utils import run_bass_kernel_spmd

    nc = _get_nc()
    in_maps = []
    for c in range(NCORES):
        b, hg = divmod(c, 4)
        hs = hg * OD
        in_maps.append({
            "xqT": _x_layout(query[:, b, :]),
            "xkT": _x_layout(keys[:, b, :]),
            "xvT": _x_layout(values[:, b, :]),
            "wqT": _w_sb_layout(Wq[hs:hs + OD, :]).astype(np.float16),
            "wkT": _w_sb_layout(Wk[hs:hs + OD, :]).astype(np.float16),
            "wvT": _w_sb_layout(Wv[hs:hs + OD, :]).astype(np.float16),
            "woT": _wo_sb_layout(Wo[:, hs:hs + OD]).astype(np.float16),
            "onesd": np.ones((128, 260), np.float32),
        })

    res = run_bass_kernel_spmd(nc, in_maps, core_ids=list(range(NCORES)),
                               trace=trace)
    global LAST_RESULT
    LAST_RESULT = res

    out = np.zeros((SL, BS, D), np.float32)
    for c in range(NCORES):
        b = c // 4
        # yT layout: [p, qc, o8, s] -> y[o8*128+p, qc*512+s]
        y = np.asarray(res.results[c]["yT"]).reshape(128, QC, 8, 512)
        out[:, b, :] += y.transpose(2, 0, 1, 3).reshape(D, SL).T.astype(np.float32)
    return out
